# revision 36
# baseline (speedup 1.0000x reference)
"""HRR self-attention (causal holographic binding) on 8 Trainium2 cores.

Math (per batch b, head h, reference semantics):
    qkv = x @ w_qkv ; q,k,v heads of HD=128
    fq,fk,fv = fft(q|k|v, axis=-1)          (length-128 FFT == matmul with DFT matrix)
    kv   = cumsum(fk*fv, axis=seq)          (causal binding)
    vals = ifft(kv * conj(fq)).real
    out  = vals @ w_out

Implementation notes:
  * The forward DFT matrices are FOLDED into the projection weights host-side
    (fq = x @ (Wq Gm), fk = x @ (Wk Gfk), fv = x @ (Wv Gm)), so the qkv
    projection directly emits packed spectra and the PE runs zero forward
    spectra matmuls.  Interleaved packing: bin j (0..63) lives in quadrant
    j//16 at R-row(j) = 32*(j//16) + j%16 (real part) and I-row(j) =
    R-row(j)+16 (imag).  I-row(0) is the DC/Nyquist helper row:
      Gm  (fq, fv): R-rows = Re bins, I-row(0) = Nyq col, I-rows = Im bins
      Gfk (fk)    : same but I-row(0) col = (1 - (-1)^a)  [= Re0 - Nyq]
  * Binding per token: kv state = (R_k*M1 + state) - (I_k*svec*M2n), one
    full-height tensor_tensor_scan, where (all partition permutes on DVE)
      R_k = shuffle(fk, [0..15 |0..15])     I_k = shuffle(fk, [16..31|16..31])
      M1  = fv                              M2n = shuffle(fv, SWAP_RI)
    svec [128,1] = +1 on R-rows, -1 on I-rows, 0 on row 0, fused into the
    pk2 product via scalar_tensor_tensor (Gm^-1 Gm2 is a signed permutation,
    so M2 = Gm2^T v is a signed row-permute of fv).  Row 16 (Nyquist) is the
    one exception: M2n is emitted as two disjoint partition-range shuffles
    (quadrant 0 uses a mask with row16->row16) so pk2[16] = fk[16]*fv[16],
    reproducing the Gfk DC/Nyquist trick exactly.
  * Unbinding: fqs = shuffle(fq, SWAP_RI); its rows 0/16 are annihilated by
    exact-zero rows of A2; vals via A1|A2 matmuls (the only spectra matmuls
    left on the PE).
  * Sharding: core c = 2*b + g handles batch b, heads 4g..4g+3.  Each core
    emits a partial out^T; the host sums the pair of partials per batch.
  * Emission is software-pipelined per head-slot s (= 4*chunk + head):
    proj(s) matmuls interleave with bind/scan DVE work of slot s-1,
    ifft(s-2), and the chunk output projection trails two slots.  PSUM is
    8 single-bank tiles (proj 3 + ifft/out 5) so the PE never waits on
    PSUM->SBUF drains.  Weight DMAs are interleaved per (k, head) with the
    first x chunk so the PE ramps with the DMA stream.
  * All matmuls fp16 (fp32 PSUM).  Folded DFT pre-scaled by 1/16; host
    undoes the net scale.
"""

import numpy as np

B, S, D, H = 4, 4096, 1024, 8
HD = 128
NCORES = 8
HPC = H // 2            # heads per core
T = 512                 # token chunk (PSUM bank = 512 fp32)
NT = S // T
KK = D // 128           # contraction tiles for the qkv projection
NSLOT = NT * HPC        # 32 head-slots
FS = 16.0               # scale folded into each forward DFT matrix
SV = 16.0               # vals stored as vals/SV
SO = 16.0               # outT stored as out/SO  (host multiplies back)


def _rrow(j):
    return 32 * (j // 16) + j % 16


def _irow(j):
    return _rrow(j) + 16


def _build_consts():
    """Forward packed DFT matrices Gm/Gfk (folded into wq host-side),
    inverse [A1|A2], and the per-partition sign vector svec."""
    n = HD
    a = np.arange(n)
    nyq = np.where(a % 2 == 0, 1.0, -1.0)              # (-1)^a

    def fwd(re_fn, i0_col, im_fn):
        M = np.zeros((n, n))
        for j in range(64):
            M[:, _rrow(j)] = re_fn(j)
        M[:, _irow(0)] = i0_col
        for j in range(1, 64):
            M[:, _irow(j)] = im_fn(j)
        return M

    def cos(j):
        return np.cos(2 * np.pi * a * j / n)

    def sin(j):
        return np.sin(2 * np.pi * a * j / n)

    Gm = fwd(cos, nyq, lambda j: -sin(j))
    Gfk = fwd(cos, cos(0) - nyq, lambda j: -sin(j))

    # inverse: vals_n = sum_p A1[p,n] P1[p] + A2[p,n] P2[p]
    A1 = np.zeros((n, n))
    A2 = np.zeros((n, n))
    for j in range(64):
        w = 1.0 if j == 0 else 2.0
        A1[_rrow(j)] = w * np.cos(2 * np.pi * j * a / n) / n
        A2[_rrow(j)] = 2.0 * np.sin(2 * np.pi * j * a / n) / n   # j=0 -> 0
    A1[_irow(0)] = nyq / n
    A2[_irow(0)] = 0.0
    for j in range(1, 64):
        A1[_irow(j)] = 2.0 * np.cos(2 * np.pi * j * a / n) / n
        A2[_irow(j)] = -2.0 * np.sin(2 * np.pi * j * a / n) / n

    Amul = FS ** 3 / SV
    amat = np.concatenate([A1 * Amul, A2 * Amul], axis=1).astype(np.float16)

    svec = np.ones((n, 1), np.float32)
    for q in range(4):
        svec[32 * q + 16:32 * q + 32, 0] = -1.0
    svec[0, 0] = 0.0
    svec[16, 0] = 1.0   # Nyquist row: pk2[16] = fk[16]*fv[16] exactly
    return Gm, Gfk, amat, svec


def _build_program():
    import concourse.bass as bass
    import concourse.bacc as bacc
    import concourse.mybir as mybir
    import concourse.tile as tile

    f16 = mybir.dt.float16
    f32 = mybir.dt.float32
    add = mybir.AluOpType.add
    sub = mybir.AluOpType.subtract
    mul = mybir.AluOpType.mult

    nc = bacc.Bacc("TRN2", target_bir_lowering=False, debug=False)
    # All DRAM tensors are pre-tiled host-side so every DMA is one
    # contiguous block.  x and wq pack PAIRS of contraction tiles so each
    # DMA descriptor row is 2 KB / 1.5 KB (DMA engines are packet-rate
    # bound; bigger rows nearly double delivery bandwidth).
    KP = KK // 2
    xT = nc.dram_tensor("xT", [KP * NT * 128, 2 * T], f16,
                        kind="ExternalInput").ap()           # [kp][t] tiles
    wq = nc.dram_tensor("wq", [KP * HPC * 128, 768], f16,
                        kind="ExternalInput").ap()           # [kp][h] tiles
    wo = nc.dram_tensor("wo", [HPC * 128, D], f16, kind="ExternalInput").ap()
    amat = nc.dram_tensor("amat", [128, 256], f16, kind="ExternalInput").ap()
    svecd = nc.dram_tensor("svec", [128, 1], f32, kind="ExternalInput").ap()
    outT = nc.dram_tensor("outT", [NT * 4 * 128, 2 * T], f16,
                          kind="ExternalOutput").ap()        # [t][od-pair] tiles

    # spect tile free-dim slices (x T columns each)
    R_, I_, M2_, FQS_ = range(4)
    DUP_R = list(range(16)) * 2          # stream_shuffle masks (per quadrant)
    DUP_I = list(range(16, 32)) * 2
    SWAP_RI = list(range(16, 32)) + list(range(16))
    # M2n quadrant-0 mask: row16 (Nyquist helper) maps to itself, so the
    # svec-signed pk2 product reproduces the Gfk DC/Nyquist trick exactly.
    Q0M = list(range(16, 32)) + [16] + list(range(1, 16))

    with tile.TileContext(nc) as tc:
        with (
            tc.tile_pool(name="consts", bufs=1) as cpool,
            tc.tile_pool(name="xin", bufs=2) as xpool,
            tc.tile_pool(name="qkvp", bufs=2) as qkvpool,
            tc.tile_pool(name="spectp", bufs=2) as spool,
            tc.tile_pool(name="pkp", bufs=2) as pkpool,
            tc.tile_pool(name="kvp", bufs=2) as kvpool,
            tc.tile_pool(name="p12p", bufs=2) as p12pool,
            tc.tile_pool(name="valp", bufs=2) as vpool,
            tc.tile_pool(name="otp", bufs=3) as otpool,
            tc.tile_pool(name="psP", bufs=3, space="PSUM") as psP,
            tc.tile_pool(name="psX", bufs=5, space="PSUM") as psX,
        ):
            # --- PE clock warm-up: the PE idles ~5us waiting for the first
            # x/wq DMAs and then ramps 0.65->1.2->2.4 GHz while running real
            # matmuls (~4us lost at reduced clock).  A chain of dummy
            # accumulations on memset data bridges the idle window so the
            # real stream starts at full clock.
            warm_sb = cpool.tile([128, T], f16, name="warm_sb")
            nc.vector.memset(warm_sb, 0.0)
            warm_ps = psX.tile([128, T], f32, tag="X", name="warm_ps")
            NWARM = 8
            for i in range(NWARM):
                nc.tensor.matmul(warm_ps, lhsT=warm_sb[:, 0:128],
                                 rhs=warm_sb, start=(i == 0),
                                 stop=(i == NWARM - 1))

            xk_tiles = {}      # (t, kp) -> [128, 2T] tile (k-pair)

            def emit_xdma(t):
                for kp in range(KP):
                    xt = xpool.tile([128, 2 * T], f16, tag=f"xk{kp}",
                                    name=f"x_{t}_{kp}")
                    r0 = (kp * NT + t) * 128
                    nc.sync.dma_start(out=xt, in_=xT[r0:r0 + 128, :])
                    xk_tiles[(t, kp)] = xt

            # weights streamed per (k-pair, head) so slot 0 can start early
            wq_sb = {}

            def emit_wq(kp, h, eng=None):
                wqt = cpool.tile([128, 768], f16, name=f"wq{kp}_{h}")
                r0 = (kp * HPC + h) * 128
                (eng or nc.sync).dma_start(out=wqt, in_=wq[r0:r0 + 128, :])
                wq_sb[(kp, h)] = wqt

            # Each dma_start fans its descriptors over all 16 DMA engines, so
            # whole tiles already move at aggregate bandwidth; keep transfers
            # coarse and split issue load across the two hwdge queues
            # (sync + scalar) so neither queue serializes the ramp.
            for kp in range(KP):
                xt = xpool.tile([128, 2 * T], f16, tag=f"xk{kp}",
                                name=f"x_0_{kp}")
                r0 = kp * NT * 128
                nc.sync.dma_start(out=xt, in_=xT[r0:r0 + 128, :])
                xk_tiles[(0, kp)] = xt
                emit_wq(kp, 0, eng=nc.scalar)
            for kp in range(KP):
                emit_wq(kp, 1, eng=nc.scalar)
            a_sb = cpool.tile([128, 256], f16, name="a_sb")
            nc.sync.dma_start(out=a_sb, in_=amat)
            sv_sb = cpool.tile([128, 1], f32, name="sv_sb")
            nc.sync.dma_start(out=sv_sb, in_=svecd)
            for h in (2, 3):
                for kp in range(KP):
                    emit_wq(kp, h)
            wo_sb = []
            for h in range(HPC):
                wot = cpool.tile([128, D], f16, name=f"wo{h}")
                nc.sync.dma_start(out=wot, in_=wo[h * 128:(h + 1) * 128, :])
                wo_sb.append(wot)

            qkv_sb = {}        # s -> [128, 3T] (fq|fk|fv)
            spect_sb = {}      # s -> [128, 4T] (R|I|M2n|fqs)
            kv_cur = {}        # h -> latest kv tile
            p12_sb = {}        # s -> [128, 2T]
            vals_sb = {}       # s -> [128, T]

            def sl(i):
                return slice(i * T, (i + 1) * T)

            def emit_proj_comp(s, comp, ps_tile):
                t, h = divmod(s, HPC)
                for k in range(KK):
                    kp, j = divmod(k, 2)
                    c0 = comp * 256 + j * 128
                    nc.tensor.matmul(
                        ps_tile,
                        lhsT=wq_sb[(kp, h)][:, c0:c0 + 128],
                        rhs=xk_tiles[(t, kp)][:, j * T:(j + 1) * T],
                        start=(k == 0),
                        stop=(k == KK - 1),
                    )

            def stage2a_q(s):
                """R_k, I_k shuffles (need qkv fk slice of s)."""
                spect_sb[s] = spool.tile([128, 4 * T], f16, tag="spect",
                                         name=f"spect_{s}")
                sp = spect_sb[s]
                qk = qkv_sb[s]
                nc.vector.stream_shuffle(sp[:, sl(R_)], qk[:, sl(1)], DUP_R)
                nc.vector.stream_shuffle(sp[:, sl(I_)], qk[:, sl(1)], DUP_I)

            def stage2a_k(s):
                """M2n/fqs shuffles + bind products (fv slice ready)."""
                sp = spect_sb[s]
                qk = qkv_sb[s]
                nc.vector.stream_shuffle(sp[:, sl(M2_)], qk[:, sl(2)],
                                         SWAP_RI)
                nc.vector.stream_shuffle(sp[0:32, sl(M2_)], qk[0:32, sl(2)],
                                         Q0M)
                nc.vector.stream_shuffle(sp[:, sl(FQS_)], qk[:, sl(0)],
                                         SWAP_RI)
                pk = pkpool.tile([128, 2 * T], f16, tag="pk", name=f"pk_{s}")
                nc.vector.tensor_mul(pk[:, 0:T], sp[:, sl(R_)], qk[:, sl(2)])
                nc.vector.scalar_tensor_tensor(
                    pk[:, T:2 * T], sp[:, sl(I_)], sv_sb, sp[:, sl(M2_)],
                    mul, mul)
                return pk

            def stage2a_v(s, pk, mul_eng=None):
                """scan + unbind muls."""
                t, h = divmod(s, HPC)
                sp = spect_sb[s]
                qk = qkv_sb[s]
                kvt = kvpool.tile([128, T], f16, tag=f"kv{h}", name=f"kv_{s}")
                init = 0.0 if t == 0 else kv_cur[h][:, T - 1:T]
                nc.vector.tensor_tensor_scan(
                    kvt, pk[:, 0:T], pk[:, T:2 * T], init, add, sub)
                kv_cur[h] = kvt
                p12 = p12pool.tile([128, 2 * T], f16, tag="p12", name=f"p12_{s}")
                # all-SBUF operands -> run the unbind muls on the idle Pool
                # engine to keep DVE clear for shuffles + the scan (the drain
                # slot passes DVE instead: Pool's slower op sits on the final
                # serial chain there)
                mul = mul_eng or nc.gpsimd.tensor_mul
                mul(p12[:, 0:T], kvt, qk[:, sl(0)])
                mul(p12[:, T:2 * T], kvt, sp[:, sl(FQS_)])
                p12_sb[s] = p12

            def stage2b(s):
                """ifft matmuls + vals copy."""
                p12 = p12_sb.pop(s)
                _, h = divmod(s, HPC)
                psval = psX.tile([128, T], f32, tag="X", name=f"psval_{s}")
                nc.tensor.matmul(psval, lhsT=a_sb[:, 0:128], rhs=p12[:, 0:T],
                                 start=True, stop=False)
                nc.tensor.matmul(psval, lhsT=a_sb[:, 128:256],
                                 rhs=p12[:, T:2 * T], start=False, stop=True)
                vt = vpool.tile([128, T], f16, tag=f"v{h}", name=f"vals_{s}")
                nc.scalar.copy(vt, psval)
                vals_sb[s] = vt

            pending_out = []   # (t, od-pair) output pieces not yet emitted

            def emit_outpiece(t, odp, e0, e1, split=False):
                """Two od tiles -> one [128, 2T] tile -> one 2KB-row DMA.
                split: drain path — store each half right after its copy so
                the final DMA only trails by half a tile."""
                ott = otpool.tile([128, 2 * T], f16, tag="ot",
                                  name=f"ot_{t}_{odp}")
                r0 = (t * 4 + odp) * 128
                for j, engine in ((0, e0), (1, e1)):
                    od = 2 * odp + j
                    ps_out = psX.tile([128, T], f32, tag="X",
                                      name=f"pso_{t}_{od}")
                    for hh in range(HPC):
                        nc.tensor.matmul(
                            ps_out,
                            lhsT=wo_sb[hh][:, od * 128:(od + 1) * 128],
                            rhs=vals_sb[t * HPC + hh],
                            start=(hh == 0),
                            stop=(hh == HPC - 1),
                        )
                    engine(ott[:, j * T:(j + 1) * T], ps_out)
                    if split:
                        nc.sync.dma_start(
                            out=outT[r0:r0 + 128, j * T:(j + 1) * T],
                            in_=ott[:, j * T:(j + 1) * T])
                if not split:
                    nc.sync.dma_start(out=outT[r0:r0 + 128, :], in_=ott)

            pk_cur = {}        # s -> pk tile awaiting scan

            for s in range(NSLOT):
                t, h = divmod(s, HPC)
                if h == 2 and t + 1 < NT:
                    emit_xdma(t + 1)
                prv = s - 1 if s >= 1 else None
                last = s == NSLOT - 1

                qkv_sb[s] = qkvpool.tile([128, 3 * T], f16, tag="qkv",
                                         name=f"qkv_{s}")
                if s == 0:
                    # slot 0 is gated by the x/wq DMA arrivals (kp order):
                    # kp-major emission lets all three components finish
                    # right as the last k-pair tile lands
                    ps3 = [psP.tile([128, T], f32, tag="P",
                                    name=f"ps{c}_{s}") for c in range(3)]
                    for k in range(KK):
                        kp, j = divmod(k, 2)
                        for comp in range(3):
                            c0 = comp * 256 + j * 128
                            nc.tensor.matmul(
                                ps3[comp],
                                lhsT=wq_sb[(kp, 0)][:, c0:c0 + 128],
                                rhs=xk_tiles[(0, kp)][:, j * T:(j + 1) * T],
                                start=(k == 0),
                                stop=(k == KK - 1),
                            )
                    for comp in range(3):
                        nc.scalar.copy(qkv_sb[s][:, comp * T:(comp + 1) * T],
                                       ps3[comp])
                    continue

                psq = psP.tile([128, T], f32, tag="P", name=f"psq_{s}")
                emit_proj_comp(s, 0, psq)
                nc.scalar.copy(qkv_sb[s][:, 0:T], psq)
                if prv is not None:
                    stage2a_q(prv)

                psk = psP.tile([128, T], f32, tag="P", name=f"psk_{s}")
                emit_proj_comp(s, 1, psk)
                nc.scalar.copy(qkv_sb[s][:, T:2 * T], psk)
                if prv is not None:
                    pk_cur[prv] = stage2a_k(prv)

                psv = psP.tile([128, T], f32, tag="P", name=f"psv_{s}")
                emit_proj_comp(s, 2, psv)
                nc.scalar.copy(qkv_sb[s][:, 2 * T:3 * T], psv)
                if prv is not None:
                    stage2a_v(prv, pk_cur.pop(prv))
                # one od-pair per slot (smooths ACT + psX load)
                if pending_out:
                    emit_outpiece(*pending_out.pop(0),
                                  nc.scalar.copy, nc.scalar.copy)
                if s >= 2:
                    stage2b(s - 2)
                    bt, bh = divmod(s - 2, HPC)
                    if bh == HPC - 1:   # chunk bt's vals complete
                        pending_out.extend((bt, odp) for odp in range(4))

                if last:   # drain the pipeline with minimal lag
                    stage2a_q(s)
                    pk_last = stage2a_k(s)
                    stage2a_v(s, pk_last, mul_eng=nc.vector.tensor_mul)
                    stage2b(s - 1)
                    # fill the PE wait on the final bind/scan chain with the
                    # leftover previous-chunk output pieces
                    for piece in pending_out:
                        emit_outpiece(*piece, nc.vector.tensor_copy,
                                      nc.scalar.copy)
                    pending_out.clear()
                    stage2b(s)
                    # alternate the final PSUM->SBUF drains between DVE and
                    # ACT so neither queue trails the PE
                    tail_engines = [
                        (nc.vector.tensor_copy, nc.scalar.copy),
                        (nc.scalar.copy, nc.vector.tensor_copy),
                        (nc.vector.tensor_copy, nc.scalar.copy),
                        (nc.scalar.copy, nc.vector.tensor_copy),
                    ]
                    for odp in range(4):
                        emit_outpiece(NT - 1, odp, *tail_engines[odp],
                                      split=(odp == 3))
    nc.compile()
    return nc


def _make_in_maps(x, w_qkv, w_out):
    Gm, Gfk, amat, svec = _build_consts()
    # Fold the forward DFT (pre-scaled by 1/FS) into the projection weights,
    # in float64, with a single fp16 rounding at the end.
    wq64 = w_qkv.astype(np.float64)
    folds = (Gm / FS, Gfk / FS, Gm / FS)    # q, k, v
    wq_fold = np.empty((D, 3 * D), np.float16)
    for comp in range(3):
        for h in range(H):
            c0 = comp * D + h * 128
            wq_fold[:, c0:c0 + 128] = (
                wq64[:, c0:c0 + 128] @ folds[comp]).astype(np.float16)
    wo16 = (w_out * (SV / SO)).astype(np.float16)
    x16 = x.astype(np.float16)
    in_maps = []
    for c in range(NCORES):
        b, g = divmod(c, 2)
        heads = range(4 * g, 4 * g + 4)
        # wq tiles [kp][h]: [128, 768] blocks with [comp][j] column order so
        # each compute component is one contiguous 256-col DMA piece
        wqt_rows = []
        for kp in range(KK // 2):
            for h in heads:
                blocks = []
                for comp in range(3):
                    for j in range(2):
                        k = 2 * kp + j
                        c0 = comp * D + h * 128
                        blocks.append(wq_fold[k * 128:(k + 1) * 128,
                                              c0:c0 + 128])
                wqt_rows.append(np.concatenate(blocks, axis=1))
        wq_tiled = np.concatenate(wqt_rows, axis=0)
        wo_rows = np.concatenate(
            [wo16[h * 128:(h + 1) * 128, :] for h in heads], axis=0)
        # x tiles [kp][t]: k-tile pairs side by side -> [128, 2T] blocks
        xt = (x16[b].T.reshape(KK // 2, 2, 128, NT, T)
              .transpose(0, 3, 2, 1, 4).reshape(KK // 2 * NT * 128, 2 * T))
        in_maps.append({
            "xT": np.ascontiguousarray(xt),
            "wq": np.ascontiguousarray(wq_tiled),
            "wo": np.ascontiguousarray(wo_rows),
            "amat": amat,
            "svec": svec,
        })
    return in_maps


_NC_CACHE = None


def _get_program():
    global _NC_CACHE
    if _NC_CACHE is None:
        _NC_CACHE = _build_program()
    return _NC_CACHE


def kernel(x, w_qkv, w_out, _trace=False, _results_out=None):
    import sys
    if "/opt/trn_rl_repo" not in sys.path:
        sys.path.insert(0, "/opt/trn_rl_repo")
    from concourse.bass_utils import run_bass_kernel_spmd

    x = np.asarray(x)
    w_qkv = np.asarray(w_qkv)
    w_out = np.asarray(w_out)
    nc = _get_program()
    in_maps = _make_in_maps(x, w_qkv, w_out)
    res = run_bass_kernel_spmd(nc, in_maps, list(range(NCORES)), trace=_trace)
    if _results_out is not None:
        _results_out.append(res)
    out = np.empty((B, S, D), np.float32)
    for b in range(B):
        p0 = res.results[2 * b]["outT"].astype(np.float32)
        p1 = res.results[2 * b + 1]["outT"].astype(np.float32)
        # outT is [t][od-pair] tiles of out^T: [NT*4*128, 2T] -> [D, S]
        oT = ((p0 + p1).reshape(NT, 4, 128, 2, T)
              .transpose(1, 3, 2, 0, 4).reshape(D, S))
        out[b] = oT.T * SO
    return out
